# revision 46
# baseline (speedup 1.0000x reference)
"""BiLSTM-CRF Trainium2 kernel (8 NeuronCores).

Topology: 8 cores = 4 batch-groups x 2 directions, 8 sequences per core.
Every core runs an identical "forward" LSTM scan (bwd cores get
time-reversed tokens). Direction pairs exchange hidden states between
layers with a pairwise AllGather; the output projection is combined with a
pairwise ReduceScatter; each core Viterbi-decodes 4 sequences (bwd cores
run the reversed DP on transposed transitions; host un-reverses).

LSTM step: gates.T [1024, 8] accumulated in PSUM on top of precomputed
x-part (done in 32-step blocks one block ahead, interleaved into PE gaps),
via 16 [128,128]x[128,8] matmuls per step (weight-load-bound).
Activations on ACT straight from PSUM; c/h updates on DVE in [128, 16]
chunk layout.
"""

import sys

sys.path.insert(0, "/opt/trn_rl_repo")

import numpy as np

V, E, H2, H, K, B, L_FULL = 50000, 256, 512, 256, 12, 32, 512
START, STOP = K - 2, K - 1
NCORES = 8
BLOC = 8
NSEQ = 4
TBLK = 32
NEG = -1.0e9

_CACHE = {}


def build_nc(L=512, n_layers=4, stage=4):
    import concourse.bass as bass
    import concourse.bacc as bacc
    import concourse.mybir as mybir
    from concourse import tile

    f32 = mybir.dt.float32
    i32 = mybir.dt.int32
    u16 = mybir.dt.uint16
    u32 = mybir.dt.uint32
    AF = mybir.ActivationFunctionType
    ALU = mybir.AluOpType

    n_blk = L // TBLK
    NT = L * BLOC
    KIN = [2] + [4] * (n_layers - 1)

    nc = bacc.Bacc("TRN2", target_bir_lowering=False, debug=False,
                   num_devices=NCORES)

    dflag = nc.declare_dram_parameter("dflag", [1, 1], u32, isOutput=False)
    emb = nc.declare_dram_parameter("emb", [V, E], f32, isOutput=False)
    tok_idx = nc.declare_dram_parameter("tok_idx", [128, NT // 128], i32,
                                        isOutput=False)
    w0 = nc.declare_dram_parameter("w0", [128, 4096], f32, isOutput=False)
    if n_layers > 1:
        wih_p = nc.declare_dram_parameter("wih", [n_layers - 1, 128, 4096],
                                          f32, isOutput=False)
        whh_p = nc.declare_dram_parameter("whh", [n_layers - 1, 128, 2048],
                                          f32, isOutput=False)
    biases_p = nc.declare_dram_parameter("biases", [n_layers, 128, 8], f32,
                                         isOutput=False)
    fcT_p = nc.declare_dram_parameter("fcT", [128, 24], f32, isOutput=False)
    fcb_p = nc.declare_dram_parameter("fc_bias", [12, 1], f32, isOutput=False)
    trep_p = nc.declare_dram_parameter("trans_rep", [128, 32], f32,
                                       isOutput=False)
    ivec_p = nc.declare_dram_parameter("init_vec", [128, 1], f32,
                                       isOutput=False)
    fvec_p = nc.declare_dram_parameter("final_vec", [128, 1], f32,
                                       isOutput=False)
    kidx_p = nc.declare_dram_parameter("kidx", [128, 1], f32, isOutput=False)
    bmask_p = nc.declare_dram_parameter("bmask", [128, 4], f32,
                                        isOutput=False)
    bassign_p = nc.declare_dram_parameter("bassign", [4, 128], f32,
                                          isOutput=False)
    ident_p = nc.declare_dram_parameter("ident", [128, 128], f32,
                                        isOutput=False)
    tags_out = nc.declare_dram_parameter("tags", [NSEQ, L], i32,
                                         isOutput=True)
    dump_out = nc.declare_dram_parameter("dump", [128, NT], f32,
                                         isOutput=True)

    with tile.TileContext(nc) as tc:
        regs = nc.alloc_registers("dflag_regs", mybir.ALL_ENGINES)
        nc.regs_load(regs, dflag[0:1, 0:1])
        sv = nc.snap(regs, donate=True, min_val=0, max_val=1)

        dramp_cm = tc.tile_pool(name="dram", bufs=1, space="DRAM")
        poolc_cm = tc.tile_pool(name="sbufc", bufs=1)
        poolw_cm = tc.tile_pool(name="sbufw", bufs=2)
        with dramp_cm as dramp, poolc_cm as poolc, poolw_cm as poolw:
            h_st = dramp.tile([2, 128, NT], f32)
            gath = dramp.tile([2, 2, 128, NT], f32)
            part_in = dramp.tile([2, NSEQ, 32, L], f32)
            feats_my = dramp.tile([NSEQ, 32, L], f32)

            # ------- constants
            ident = poolc.tile([128, 128], f32, tag="ident", name="ident")
            nc.sync.dma_start(out=ident[:], in_=ident_p[:])
            w0_sb = poolc.tile([128, 4096], f32, tag="w0", name="w0")
            nc.sync.dma_start(out=w0_sb[:], in_=w0[:])
            bias_sb = poolc.tile([128, 8 * n_layers], f32, tag="bias", name="bias")
            for l in range(n_layers):
                nc.sync.dma_start(out=bias_sb[:, 8 * l:8 * l + 8],
                                  in_=biases_p[l])

            # ------- embedding gather + transpose into layer-0 x chunks
            x_own = [poolw.tile([128, NT], f32, tag=f"x_own{k}", name=f"x_own{k}")
                     for k in range(2)]
            idx_all = poolc.tile([128, NT // 128], i32, tag="idx_all",
                                 name="idx_all")
            nc.sync.dma_start(out=idx_all[:], in_=tok_idx[:])
            with tc.tile_pool(name="psum_e", bufs=2, space="PSUM") as ppe:
                for j in range(NT // 128):
                    gt = poolw.tile([128, 256], f32, tag="gath_t", name="gath_t")
                    nc.gpsimd.indirect_dma_start(
                        out=gt[:], out_offset=None, in_=emb[:],
                        in_offset=bass.IndirectOffsetOnAxis(
                            ap=idx_all[:, j:j + 1], axis=0))
                    for k in range(2):
                        pt = ppe.tile([128, 128], f32, tag="pe_tr", name="pe_tr")
                        nc.tensor.transpose(pt[:],
                                            gt[:, 128 * k:128 * k + 128],
                                            ident[:])
                        nc.vector.tensor_copy(
                            x_own[k][:, 128 * j:128 * j + 128], pt[:])

            if stage == 1:
                nc.sync.dma_start(out=dump_out[:], in_=x_own[0][:])
                nc.compile._noop if False else None
            # ------- LSTM layers
            x_cur = x_own
            partner = None
            x_next = None

            with tc.tile_pool(name="psum_g", bufs=2, space="PSUM") as ppg:
                for l in range(n_layers if stage >= 2 else 0):
                    kin = KIN[l]
                    if l == 0:
                        wih_sb, whh_sb = w0_sb, w0_sb
                        whh_off = 2048
                    else:
                        wih_sb = poolc.tile([128, 4096], f32, tag="wih", name="wih")
                        nc.sync.dma_start(out=wih_sb[:], in_=wih_p[l - 1])
                        whh_sb = poolc.tile([128, 2048], f32, tag="whh", name="whh")
                        nc.sync.dma_start(out=whh_sb[:], in_=whh_p[l - 1])
                        whh_off = 0
                    bias_ap = bias_sb[:, 8 * l:8 * l + 8]

                    x_next = [poolw.tile([128, NT], f32, tag=f"x_own{k}", name=f"x_own{k}")
                              for k in range(2)]

                    def xrhs(k, c0, cn, l=l):
                        if k < 2:
                            return x_cur[k][:, c0:c0 + cn]
                        src = partner[k - 2][:].rearrange(
                            "p (t b) -> p t b", b=8)
                        t0 = c0 // 8
                        tn = cn // 8
                        hi = L - 1 - t0
                        lo = L - t0 - tn
                        if lo == 0:
                            return src[:, hi::-1, :]
                        return src[:, hi:lo - 1:-1, :]

                    def new_banks():
                        return [ppg.tile([128, 512], f32, tag=f"gate_b{j}", name=f"gate_b{j}")
                                for j in range(4)]

                    def precompute_ops(blk, banks, kin=kin, bias_ap=bias_ap,
                                       wih_sb=wih_sb):
                        c0 = TBLK * 8 * blk
                        for m in range(8):
                            bank = banks[m // 2]
                            r0 = 256 * (m % 2)
                            for k in range(kin):
                                col = (k * 8 + m) * 128

                                def op(m=m, k=k, bank=bank, r0=r0, col=col):
                                    nc.tensor.matmul(
                                        bank[:, r0:r0 + 256],
                                        wih_sb[:, col:col + 128],
                                        xrhs(k, c0, 256),
                                        start=(k == 0 and m % 2 == 0),
                                        stop=False,
                                        skip_group_check=True)
                                yield op

                            def bop(m=m, bank=bank, r0=r0, bias_ap=bias_ap):
                                nc.scalar.add(bank[:, r0:r0 + 256],
                                              bank[:, r0:r0 + 256],
                                              bias_ap[:, m:m + 1])
                            yield bop

                    h_init = poolc.tile([128, 16], f32, tag="h_init", name="h_init")
                    nc.vector.memset(h_init[:], 0.0)
                    c_prev = poolw.tile([128, 16], f32, tag="c", name="c")
                    nc.vector.memset(c_prev[:], 0.0)

                    banks_cur = new_banks()
                    for op in precompute_ops(0, banks_cur):
                        op()
                    h_blk_prev = None
                    GATE_MS = (("g", (4, 5)), ("i", (0, 1)), ("f", (2, 3)),
                               ("o", (6, 7)))
                    ACOL = {"i": 0, "f": 16, "g": 32, "o": 48}

                    for blk in range(n_blk):
                        if blk + 1 < n_blk:
                            banks_next = new_banks()
                            pre_iter = precompute_ops(blk + 1, banks_next)
                        else:
                            banks_next = None
                            pre_iter = iter(())
                        h_blk = poolw.tile([128, 512], f32, tag="h_blk", name="h_blk")
                        for s_l in range(TBLK):
                            if s_l == 0 and blk == 0:
                                hsrc, hc0 = h_init, None
                            elif s_l == 0:
                                hsrc, hc0 = h_blk_prev, 8 * (TBLK - 1)
                            else:
                                hsrc, hc0 = h_blk, 8 * (s_l - 1)

                            act = poolw.tile([128, 64], f32, tag="act", name="act")
                            gcol = 8 * s_l
                            for gate, ms in GATE_MS:
                                for m in ms:
                                    bank = banks_cur[m // 2]
                                    r0 = 256 * (m % 2)
                                    for k in range(2):
                                        col = whh_off + (k * 8 + m) * 128
                                        if hc0 is None:
                                            hr = h_init[:, 8 * k:8 * k + 8]
                                        else:
                                            hb = 256 * k + hc0
                                            hr = hsrc[:, hb:hb + 8]
                                        nc.tensor.matmul(
                                            bank[:, r0 + gcol:r0 + gcol + 8],
                                            whh_sb[:, col:col + 128], hr,
                                            start=False, stop=(k == 1),
                                            skip_group_check=True)
                                j = ms[0] // 2
                                pair = banks_cur[j][:].rearrange(
                                    "p (r c) -> p r c", r=2)[:, :,
                                                             gcol:gcol + 8]
                                func = (AF.Tanh if gate == "g"
                                        else AF.Sigmoid)
                                acol = ACOL[gate]
                                nc.scalar.activation(act[:, acol:acol + 16],
                                                     pair, func)
                                nxt = next(pre_iter, None)
                                if nxt is not None:
                                    nxt()
                            tmp1 = poolw.tile([128, 16], f32, tag="tmp1", name="tmp1")
                            nc.vector.tensor_mul(tmp1[:], act[:, 0:16],
                                                 act[:, 32:48])
                            tmp2 = poolw.tile([128, 16], f32, tag="tmp2", name="tmp2")
                            nc.vector.tensor_mul(tmp2[:], act[:, 16:32],
                                                 c_prev[:])
                            c_new = poolw.tile([128, 16], f32, tag="c", name="c")
                            nc.vector.tensor_add(c_new[:], tmp1[:], tmp2[:])
                            tc_t = poolw.tile([128, 16], f32, tag="tanh_c", name="tanh_c")
                            nc.scalar.activation(tc_t[:], c_new[:], AF.Tanh)
                            h_ap = h_blk[:].rearrange(
                                "p (r c) -> p r c", r=2)[:, :, gcol:gcol + 8]
                            nc.vector.tensor_mul(h_ap, act[:, 48:64],
                                                 tc_t[:])
                            c_prev = c_new
                        for k in range(2):
                            d0 = 256 * blk
                            nc.gpsimd.tensor_copy(x_next[k][:, d0:d0 + 256],
                                             h_blk[:, 256 * k:256 * k + 256])
                        h_blk_prev = h_blk
                        banks_cur = banks_next

                    if l < n_layers - 1:
                        for k in range(2):
                            nc.sync.dma_start(out=h_st[k], in_=x_next[k][:])
                        nc.gpsimd.collective_compute(
                            "AllGather", ALU.bypass,
                            replica_groups=[[0, 1], [2, 3], [4, 5], [6, 7]],
                            ins=[h_st[:]], outs=[gath[:]])
                        partner = [poolc.tile([128, NT], f32, tag=f"pr{k}", name=f"pr{k}")
                                   for k in range(2)]
                        with tc.If(sv == 1) as cmp:
                            for k in range(2):
                                nc.sync.dma_start(out=partner[k][:],
                                                  in_=gath[0, k])
                        with cmp.Else():
                            for k in range(2):
                                nc.sync.dma_start(out=partner[k][:],
                                                  in_=gath[1, k])
                        x_cur = x_next

            if stage == 2:
                nc.sync.dma_start(out=dump_out[:], in_=x_next[0][:])
            if stage >= 3:
                # ------- feats partials (written b-major, natural + reversed)
                fcT_sb = poolc.tile([128, 24], f32, tag="fcT", name="fcT")
                nc.sync.dma_start(out=fcT_sb[:], in_=fcT_p[:])
                fcb_sb = poolc.tile([12, 1], f32, tag="fcb", name="fcb")
                nc.sync.dma_start(out=fcb_sb[:], in_=fcb_p[:])
                pnat = poolc.tile([12, NT], f32, tag="pr0", name="pnat")
                prev = poolc.tile([12, NT], f32, tag="pr1", name="prev")
                pnat_tb = pnat[:].rearrange("p (b t) -> p t b", t=L)
                prev_tb = prev[:].rearrange("p (b t) -> p t b", t=L)[:, ::-1, :]
                FCH = min(512, NT)
                TL = FCH // 8
                with tc.tile_pool(name="psum_f", bufs=2, space="PSUM") as ppf:
                    for nb in range(NT // FCH):
                        ps = ppf.tile([12, FCH], f32, tag="feat", name="feat")
                        for k in range(2):
                            nc.tensor.matmul(
                                ps[:], fcT_sb[:, 12 * k:12 * k + 12],
                                x_next[k][:, FCH * nb:FCH * nb + FCH],
                                start=(k == 0), stop=(k == 1))
                        ps3 = ps[:].rearrange("p (t b) -> p t b", b=8)
                        t0 = nb * TL
                        nc.scalar.add(pnat_tb[:, t0:t0 + TL, :], ps3,
                                      fcb_sb[:, 0:1])
                        nc.scalar.add(prev_tb[:, t0:t0 + TL, :], ps3,
                                      fcb_sb[:, 0:1])

                # pad rows of part_in must be zero (summed by ReduceScatter)
                zrows = poolc.tile([32, L], f32, tag="zrows", name="zrows")
                nc.vector.memset(zrows[:], 0.0)
                for h in range(2):
                    for bq in range(NSEQ):
                        nc.sync.dma_start(out=part_in[h, bq, 12:32],
                                          in_=zrows[0:20, :])

                def write_half(half, bs, buf):
                    b3 = buf[:].rearrange("p (b t) -> p b t", t=L)
                    for bq in range(4):
                        nc.sync.dma_start(out=part_in[half, bq, 0:12],
                                          in_=b3[:, bs + bq, :])

                with tc.If(sv == 1) as cmp:
                    write_half(0, 0, prev)
                    write_half(1, 4, pnat)
                with cmp.Else():
                    write_half(0, 0, pnat)
                    write_half(1, 4, prev)

                nc.gpsimd.collective_compute(
                    "ReduceScatter", ALU.add,
                    replica_groups=[[0, 1], [2, 3], [4, 5], [6, 7]],
                    ins=[part_in[:]], outs=[feats_my[:]])

                if stage == 3:
                    nc.sync.dma_start(out=dump_out[0:12, :], in_=pnat[:])
            if stage >= 4:
                # ------- Viterbi forward
                featsT = poolc.tile([128, L], f32, tag="featsT", name="featsT")
                nc.sync.dma_start(
                    out=featsT[:],
                    in_=feats_my[:].rearrange("b k l -> (b k) l"))

                trep = poolc.tile([128, 32], f32, tag="trep", name="trep")
                nc.sync.dma_start(out=trep[:], in_=trep_p[:])
                ivec = poolc.tile([128, 1], f32, tag="ivec", name="ivec")
                nc.sync.dma_start(out=ivec[:], in_=ivec_p[:])
                fvec = poolc.tile([128, 1], f32, tag="fvec", name="fvec")
                nc.sync.dma_start(out=fvec[:], in_=fvec_p[:])
                kidx_sb = poolc.tile([128, 1], f32, tag="kidx", name="kidx")
                nc.sync.dma_start(out=kidx_sb[:], in_=kidx_p[:])
                bmask_sb = poolc.tile([128, 4], f32, tag="bmask", name="bmask")
                nc.sync.dma_start(out=bmask_sb[:], in_=bmask_p[:])
                bassign_sb = poolc.tile([4, 128], f32, tag="bassign", name="bassign")
                nc.sync.dma_start(out=bassign_sb[:], in_=bassign_p[:])

                bp8 = poolc.tile([128, 8 * L], u16, tag="bp8", name="bp8")
                nc.vector.memset(bp8[:, 0:8], 0)

                score = poolw.tile([128, 1], f32, tag="score", name="score")
                nc.vector.tensor_add(score[:], featsT[:, 0:1], ivec[:])
                for s in range(1, L):
                    cand = poolw.tile([128, 32], f32, tag="cand", name="cand")
                    nc.vector.tensor_scalar_add(cand[:], trep[:], score[:, 0:1])
                    candT = poolw.tile([128, 32], f32, tag="candT", name="candT")
                    nc.vector.transpose(candT[:], cand[:])
                    mx = poolw.tile([128, 8], f32, tag="mx", name="mx")
                    nc.vector.max(mx[:], candT[:])
                    nc.vector.max_index(bp8[:, 8 * s:8 * s + 8], mx[:], candT[:])
                    score = poolw.tile([128, 1], f32, tag="score", name="score")
                    nc.vector.tensor_scalar_add(score[:], mx[:, 0:1],
                                                featsT[:, s:s + 1])
                score2 = poolw.tile([128, 1], f32, tag="score", name="score")
                nc.vector.tensor_add(score2[:], score[:], fvec[:])

                # endgame: onehot of per-group argmax
                zeros32 = poolc.tile([128, 32], f32, tag="z32", name="z32")
                nc.vector.memset(zeros32[:], 0.0)
                sc_sp = poolw.tile([128, 32], f32, tag="cand", name="cand")
                nc.vector.tensor_scalar_add(sc_sp[:], zeros32[:], score2[:, 0:1])
                scT = poolw.tile([128, 32], f32, tag="candT", name="candT")
                nc.vector.transpose(scT[:], sc_sp[:])
                maxrep = poolw.tile([128, 1], f32, tag="maxrep", name="maxrep")
                nc.vector.reduce_max(maxrep[:], scT[:],
                                     axis=mybir.AxisListType.X)
                oh0 = poolw.tile([128, 1], f32, tag="oh", name="oh")
                nc.vector.tensor_tensor(out=oh0[:], in0=score2[:], in1=maxrep[:],
                                        op=ALU.is_equal)

                tags_f = poolc.tile([4, L], f32, tag="tags_f", name="tags_f")
                bp_all = poolc.tile([128, L], f32, tag="bp_all", name="bp_all")
                nc.vector.tensor_copy(bp_all[:], bp8[:, 0:8 * L:8])

                with tc.tile_pool(name="psum_v", bufs=2, space="PSUM") as ppv:
                    def bt_step(oh_ap, rhs_ap, s):
                        ohm = poolw.tile([128, 4], f32, tag="ohm", name="ohm")
                        nc.vector.tensor_scalar_mul(ohm[:], bmask_sb[:],
                                                    oh_ap[:, 0:1])
                        psA = ppv.tile([4, 1], f32, tag="vA", name="vA")
                        nc.tensor.matmul(psA[:], ohm[:], rhs_ap, start=True,
                                         stop=True)
                        nc.vector.tensor_copy(tags_f[:, s:s + 1], psA[:])
                        psB = ppv.tile([128, 1], f32, tag="vB", name="vB")
                        nc.tensor.matmul(psB[:], bassign_sb[:],
                                         tags_f[:, s:s + 1], start=True,
                                         stop=True)
                        srep = poolw.tile([128, 1], f32, tag="strep", name="strep")
                        nc.vector.tensor_copy(srep[:], psB[:])
                        return srep

                    state_rep = bt_step(oh0, kidx_sb[:, 0:1], L - 1)
                    for s in range(L - 2, -1, -1):
                        ohx = poolw.tile([128, 1], f32, tag="oh", name="oh")
                        nc.vector.tensor_tensor(out=ohx[:], in0=state_rep[:],
                                                in1=kidx_sb[:],
                                                op=ALU.is_equal)
                        state_rep = bt_step(ohx, bp_all[:, s + 1:s + 2], s)

                tags_i = poolc.tile([4, L], i32, tag="tags_i", name="tags_i")
                nc.vector.tensor_copy(tags_i[:], tags_f[:])
                nc.sync.dma_start(out=tags_out[:], in_=tags_i[:])

    nc.compile()
    return nc


def _finish(nc):
    return nc


# ---------------------------------------------------------------------------
# host side
# ---------------------------------------------------------------------------

def _tiles_T(W, kin):
    """W [1024, 128*kin] -> [128, kin*8*128] lhsT tile layout,
    col (k*8+m)*128+q = W[128m+q, 128k+p] at partition p."""
    return np.ascontiguousarray(
        W.reshape(8, 128, kin, 128).transpose(3, 2, 0, 1).reshape(128, -1))


def _prep_core(c, inp, L, n_layers):
    g, d = c // 2, c % 2
    tok = np.asarray(inp["tokens"])[8 * g:8 * g + 8, :L]
    if d == 1:
        tok = tok[:, ::-1]
    NT = L * BLOC
    idx = np.ascontiguousarray(
        tok.T.reshape(NT).reshape(NT // 128, 128).T.astype(np.int32))

    w_ih0 = np.asarray(inp["w_ih0"])
    w_hh0 = np.asarray(inp["w_hh0"])
    b0 = np.asarray(inp["b0"])
    w_ih = np.asarray(inp["w_ih"])
    w_hh = np.asarray(inp["w_hh"])
    bb = np.asarray(inp["b"])
    fc_w = np.asarray(inp["fc_w"])
    fc_b = np.asarray(inp["fc_b"])
    trans = np.asarray(inp["transitions"])

    w0 = np.concatenate([_tiles_T(w_ih0[d], 2), _tiles_T(w_hh0[d], 2)],
                        axis=1)
    wihs, whhs = [], []
    bs = [b0[d].reshape(8, 128).T]
    for l in range(n_layers - 1):
        Wl = w_ih[l, d]
        own = Wl[:, 256 * d:256 * d + 256]
        oth = Wl[:, 256 * (1 - d):256 * (1 - d) + 256]
        wihs.append(_tiles_T(np.concatenate([own, oth], axis=1), 4))
        whhs.append(_tiles_T(w_hh[l, d], 2))
        bs.append(bb[l, d].reshape(8, 128).T)

    fch = fc_w[:, 256 * d:256 * d + 256]
    fcT = np.ascontiguousarray(
        fch.T.reshape(2, 128, 12).transpose(1, 0, 2).reshape(128, 24))
    fcb = (fc_b if d == 0 else np.zeros(12, np.float32)).reshape(12, 1)

    Tc = trans if d == 0 else trans.T
    trep = np.full((128, 32), NEG, np.float32)
    ivec = np.full((128, 1), NEG, np.float32)
    fvec = np.zeros((128, 1), np.float32)
    for bq in range(4):
        trep[32 * bq:32 * bq + 12, 0:12] = Tc
        if d == 0:
            ivec[32 * bq:32 * bq + 12, 0] = trans[START, :]
            fvec[32 * bq:32 * bq + 12, 0] = trans[:, STOP]
        else:
            ivec[32 * bq:32 * bq + 12, 0] = trans[:, STOP]
            fvec[32 * bq:32 * bq + 12, 0] = trans[START, :]

    kidx = (np.arange(128) % 32).astype(np.float32).reshape(128, 1)
    bmask = np.zeros((128, 4), np.float32)
    bassign = np.zeros((4, 128), np.float32)
    for bq in range(4):
        bmask[32 * bq:32 * bq + 32, bq] = 1.0
        bassign[bq, 32 * bq:32 * bq + 32] = 1.0

    d_in = {
        "dflag": np.array([[d]], np.uint32),
        "emb": np.asarray(inp["embed"], np.float32),
        "tok_idx": idx,
        "w0": np.ascontiguousarray(w0, np.float32),
        "biases": np.ascontiguousarray(np.stack(bs), np.float32),
        "fcT": np.ascontiguousarray(fcT, np.float32),
        "fc_bias": np.ascontiguousarray(fcb, np.float32),
        "trans_rep": trep,
        "init_vec": ivec,
        "final_vec": fvec,
        "kidx": kidx,
        "bmask": bmask,
        "bassign": bassign,
        "ident": np.eye(128, dtype=np.float32),
    }
    if n_layers > 1:
        d_in["wih"] = np.ascontiguousarray(np.stack(wihs), np.float32)
        d_in["whh"] = np.ascontiguousarray(np.stack(whhs), np.float32)
    return d_in


def get_nc(L=512, n_layers=4, stage=4):
    key = (L, n_layers, stage)
    if key not in _CACHE:
        _CACHE[key] = build_nc(L, n_layers, stage)
    return _CACHE[key]


def run_on_hw(inputs, L=512, n_layers=4, stage=4, raw=False):
    from concourse.bass_utils import run_bass_kernel_spmd

    nc = get_nc(L, n_layers, stage)
    in_maps = [_prep_core(c, inputs, L, n_layers) for c in range(NCORES)]
    res = run_bass_kernel_spmd(nc, in_maps, list(range(NCORES)))
    if raw:
        return res
    out = np.zeros((B, L), np.int32)
    for c in range(NCORES):
        g, d = c // 2, c % 2
        tags = res.results[c]["tags"]
        if d == 0:
            out[8 * g:8 * g + 4] = tags
        else:
            out[8 * g + 4:8 * g + 8] = tags[:, ::-1]
    return out


def kernel(**inputs):
    return run_on_hw(inputs, 512, 4)

